# revision 18
# baseline (speedup 1.0000x reference)
"""CapsuleCONV Trainium2 kernel (nn_CapsuleCONV_1709396984016).

Math (per batch b):
  unfold input [N,32,32,16] with K=3,stride=2 -> patches X[n,k,l,hw,a,x]
  votes V[n,kl,hw,a,(d,m)] = sum_x X * w[k,l,n,x,d,m]
  logits qk[n,kl,m,hw] = 0.25 * sum_{a,d} V * ncv[b,m,hw,a,d]
  qk = softmax_m(qk);  out[m,hw,a,d] = sum_{n,kl} qk * V
  out = LayerNorm_{(a,d)}(out) * gamma + beta

Device mapping (8 cores, 4 batches each):
  partitions p = 4n+x = 32i+4g+x   (n = 8i+g)
  votes matmul: 16-tile-packed block-diagonal lhsT tiles (K=32 = 8 caps x
  4 pose cols on the diagonal), rhs = pose columns of the input slab.
  dm slot of PSUM partition (j,g,c4) at round r:  d=j, m=4r+c4.
  Selection matmuls (host-built 0/1 matrices) do all partition-dim sums:
  SQ (sum_j -> logits), SD (sum_{m} -> softmax denom, replicated),
  SN_r (sum_g, unscramble to 4m+d), SM (sum_d for LN, replicated),
  Sd4 (split d for the output permute).
"""
import os
import numpy as np
import ml_dtypes
from contextlib import ExitStack

import concourse.bass as bass
import concourse.tile as tile
from concourse import bacc, mybir
from concourse._compat import with_exitstack

F32 = mybir.dt.float32
BF16 = mybir.dt.bfloat16
BF = ml_dtypes.bfloat16

B, N, H, W, DIN = 32, 32, 32, 32, 16
M, DOUT = 32, 16
KK, STRIDE = 3, 2
HO = WO = 15
HWO = HO * WO  # 225
NCORES = 8
NB = B // NCORES  # 4 batches per core
SCALE = 0.25      # 1/sqrt(16)
LN_EPS = 1e-5
# h-halves for 512-elem PSUM banks: h rows [0,8) and [8,15)
HALVES = [(0, 8), (8, 7)]
# hw halves for the output accumulator (<=128 free per a)
OHALVES = [(0, 113), (113, 112)]


# ---------------------------------------------------------------- host prep
def host_prep(input_, ncv, w, gamma, beta):
    # [B, (n,x), (a, H, W)] — a OUTER of (h,w) so PSUM banks drain
    # contiguously (bank free = (a, hw) with hw innermost)
    inputT = np.ascontiguousarray(
        input_.reshape(B, N, H, W, 4, 4).transpose(0, 1, 5, 4, 2, 3)
        .reshape(B, 128, 4 * H * W)).astype(BF)

    # U_scr[b, 32j+4g+c4, a, r, hw] = ncv[b, 4r+c4, hw, a*4+j]
    nc6 = ncv.reshape(B, 8, 4, HWO, 4, 4)        # [b, r, c4, hw, a, d]
    u = nc6.transpose(0, 5, 2, 4, 1, 3)          # [b, d(=j), c4, a, r, hw]
    u = np.broadcast_to(u[:, :, None, :, :, :, :],
                        (B, 4, 8, 4, 4, 8, HWO))  # + g
    # -> [b, (j,g,c4), a, r, hw]
    u_scr = np.ascontiguousarray(
        u.transpose(0, 1, 2, 3, 4, 5, 6).reshape(B, 128, 4 * 8 * HWO)
    ).astype(BF)

    # w_bd[kl, r, 32i+4g+x, 32j+4g+c4] = w[k,l, 8i+g, x, j, 4r+c4]
    v = w.reshape(9, 4, 8, 4, 4, 8, 4)           # [kl, i, g, x, d(=j), r, c4]
    v = v.transpose(0, 5, 1, 2, 3, 4, 6)         # [kl, r, i, g, x, j, c4]
    eye8 = np.eye(8, dtype=np.float32)
    wbd = np.einsum('qrigxjc,gh->qrigxjhc', v, eye8)  # [kl,r,i,g,x,j,g',c4]
    w_bd = np.ascontiguousarray(wbd.reshape(9, 8, 128, 128)).astype(BF)

    # SQ[(j,g,c4), (g',c4')] = 0.25 * d_{g,g'} d_{c4,c4'}
    sq = np.zeros((128, 32), np.float32)
    for j in range(4):
        for g in range(8):
            for c in range(4):
                sq[32 * j + 4 * g + c, 4 * g + c] = SCALE
    # SD3[(g,c4), (g',c4')] = d_{g,g'}  (sum over c4', replicate over c4)
    sd3 = np.zeros((32, 32), np.float32)
    for g in range(8):
        for c in range(4):
            for c2 in range(4):
                sd3[4 * g + c, 4 * g + c2] = 1.0
    # SN[r][(j,g,c4), 4*(4r+c4)+j] = 1   (sum over g)
    sn = np.zeros((8, 128, 128), np.float32)
    for r in range(8):
        for j in range(4):
            for g in range(8):
                for c in range(4):
                    sn[r, 32 * j + 4 * g + c, 4 * (4 * r + c) + j] = 1.0
    # SM[(4m+d), (4m'+d')] = d_{m,m'} / 16
    sm = np.zeros((128, 128), np.float32)
    for m in range(32):
        for d in range(4):
            for d2 in range(4):
                sm[4 * m + d, 4 * m + d2] = 1.0 / 16.0
    # Sd4[d][(4m+dd), m'] = d_{m,m'} d_{dd,d}
    sd4 = np.zeros((4, 128, 32), np.float32)
    for d in range(4):
        for m in range(32):
            sd4[d, 4 * m + d, m] = 1.0
    # per-partition gamma/beta columns: gcols[p, a] = gamma[a*4 + p%4]
    gcols = np.zeros((128, 4), np.float32)
    bcols = np.zeros((128, 4), np.float32)
    for p in range(128):
        for a in range(4):
            gcols[p, a] = gamma[a * 4 + p % 4]
            bcols[p, a] = beta[a * 4 + p % 4]

    return dict(inputT=inputT, u_scr=u_scr, w_bd=w_bd,
                sq=sq.astype(BF), sd3=sd3.astype(BF), sn=sn.astype(BF),
                sm=sm, sd4=sd4, gcols=gcols, bcols=bcols)


# ------------------------------------------------------------- tile program
@with_exitstack
def build_program(ctx: ExitStack, tc: tile.TileContext, dram: dict,
                  nb=NB, kls=None, stage=4):
    nc = tc.nc
    if kls is None:
        kls = [(k, l) for k in range(3) for l in range(3)]

    const = ctx.enter_context(tc.tile_pool(name="const", bufs=1))
    vpool = ctx.enter_context(tc.tile_pool(name="vpool", bufs=3))
    xpool = ctx.enter_context(tc.tile_pool(name="xpool", bufs=2))
    upool = ctx.enter_context(tc.tile_pool(name="upool", bufs=1))
    ppool = ctx.enter_context(tc.tile_pool(name="ppool", bufs=2))
    p2pool = ctx.enter_context(tc.tile_pool(name="p2pool", bufs=2))
    spool = ctx.enter_context(tc.tile_pool(name="spool", bufs=1))
    mpool = ctx.enter_context(tc.tile_pool(name="mpool", bufs=2))
    vps = ctx.enter_context(tc.tile_pool(name="vps", bufs=2, space="PSUM"))
    qps = ctx.enter_context(tc.tile_pool(name="qps", bufs=2, space="PSUM"))
    ops = ctx.enter_context(tc.tile_pool(name="ops", bufs=1, space="PSUM"))

    # constants
    wbd_sb = const.tile([128, 9 * 8 * 128], BF16, tag="wbd")
    nc.sync.dma_start(wbd_sb[:].rearrange("p (q r c) -> p q r c", q=9, r=8),
                      dram["w_bd"].rearrange("q r p c -> p q r c"))
    sq_sb = const.tile([128, 32], BF16, tag="sq")
    nc.sync.dma_start(sq_sb[:], dram["sq"])
    sd3_sb = const.tile([32, 32], BF16, tag="sd3")
    nc.sync.dma_start(sd3_sb[:], dram["sd3"])
    sn_sb = const.tile([128, 8 * 128], BF16, tag="sn")
    nc.sync.dma_start(sn_sb[:].rearrange("p (r c) -> p r c", r=8),
                      dram["sn"].rearrange("r p c -> p r c"))
    sm_sb = const.tile([128, 128], F32, tag="sm")
    nc.sync.dma_start(sm_sb[:], dram["sm"])
    sd4_sb = const.tile([128, 4 * 32], F32, tag="sd4")
    nc.sync.dma_start(sd4_sb[:].rearrange("p (d c) -> p d c", d=4),
                      dram["sd4"].rearrange("d p c -> p d c"))
    gc_sb = const.tile([128, 4], F32, tag="gc")
    nc.sync.dma_start(gc_sb[:], dram["gcols"])
    bc_sb = const.tile([128, 4], F32, tag="bc")
    nc.sync.dma_start(bc_sb[:], dram["bcols"])
    eps_sb = const.tile([128, 1], F32, tag="eps")
    nc.vector.memset(eps_sb[:], LN_EPS)

    wbd_v = wbd_sb[:].rearrange("p (q r c) -> p q r c", q=9, r=8)

    for bi in range(nb):
        xt = xpool.tile([128, 4096], BF16, tag="xt")
        nc.sync.dma_start(xt[:], dram["inputT"][bi])
        xt_v = xt[:].rearrange("p (a h w) -> p a h w", a=4, h=32)
        u_sb = upool.tile([128, 4 * 8 * HWO], BF16, tag="u")
        nc.sync.dma_start(u_sb[:], dram["u_scr"][bi])

        # output accumulator: free = (a, hw-half)
        if stage >= 4:
            oacc = [ops.tile([128, 512], F32, tag=f"oacc{h}", name=f"oacc{h}")
                    [:, :4 * OHALVES[h][1]] for h in range(2)]
        first_acc = [True, True]
        dcnt = [0]

        def emit_votes(k, l, kl, i):
            """Votes for one (kl, i): r-paired 2-bank PSUM tiles, one
            contiguous drain per pair, alternating Scalar/GpSimd."""
            votes_t = vpool.tile([128, 4 * 8 * HWO], BF16, tag="votes")
            votes_v = votes_t[:].rearrange("p (a r hw) -> p a r hw", a=4, r=8)
            for rp in range(4):
                for hx, (h0, nh) in enumerate(HALVES):
                    pair = vps.tile([128, 1024], F32, tag="vb",
                                    name=f"vb{hx}")
                    for e in range(2):
                        r = 2 * rp + e
                        nc.tensor.matmul(
                            pair[:, 512 * e:512 * e + nh * 60],
                            wbd_v[32 * i:32 * i + 32, kl, r, :],
                            xt_v[32 * i:32 * i + 32, :,
                                 k + 2 * h0:k + 2 * (h0 + nh):2,
                                 l:l + 29:2],
                            start=True, stop=True,
                            tile_position=(32 * i, 0),
                        )
                    if stage >= 1:
                        src = (pair[:].rearrange("p (b x) -> p b x", b=2)
                               [:, :, :nh * 60]
                               .rearrange("p b (a q) -> p a b q", a=4))
                        dst = votes_v[:, :, 2 * rp:2 * rp + 2,
                                      h0 * 15:(h0 + nh) * 15]
                        if dcnt[0] % 4 == 3:
                            nc.vector.tensor_copy(dst, src)
                        else:
                            nc.scalar.copy(dst, src)
                        dcnt[0] += 1
            return votes_t

        def consume_front(votes_t, i):
            """P-mul, logits matmuls, exp, and the gpsimd denom tree."""
            if stage < 2:
                return None
            # ---- P = votes * U  (one TT, bf16 2x)
            pt = ppool.tile([128, 4 * 8 * HWO], BF16, tag="pt")
            nc.vector.tensor_mul(pt[:], votes_t[:], u_sb[:])
            pt_v = pt[:].rearrange("p (a r hw) -> p a r hw", a=4, r=8)

            # ---- logits: sum_j via SQ (r-pairs in free), sum_a accumulated
            e_sb = spool.tile([32, 8 * HWO], BF16, tag="e", bufs=2,
                              name=f"e{i}")
            for r2 in range(4):
                qk = qps.tile([32, 512], F32, tag="qk",
                              name="qk")[:, :2 * HWO]
                for a in range(4):
                    nc.tensor.matmul(
                        qk[:, :],
                        sq_sb[:],
                        pt_v[:, a, 2 * r2:2 * r2 + 2, :],
                        start=(a == 0), stop=(a == 3),
                    )
                nc.scalar.activation(
                    e_sb[:, 2 * r2 * HWO:(2 * r2 + 2) * HWO], qk[:],
                    mybir.ActivationFunctionType.Exp)

            # ---- softmax denom partial: sum_r tree on gpsimd
            et = mpool.tile([32, 6 * HWO], BF16, tag="esum")
            nc.gpsimd.tensor_add(et[:, :4 * HWO], e_sb[:, :4 * HWO],
                                 e_sb[:, 4 * HWO:])
            nc.gpsimd.tensor_add(et[:, 4 * HWO:6 * HWO], et[:, :2 * HWO],
                                 et[:, 2 * HWO:4 * HWO])
            esum = mpool.tile([32, HWO], BF16, tag="esum2")
            nc.gpsimd.tensor_add(esum[:], et[:, 4 * HWO:5 * HWO],
                                 et[:, 5 * HWO:6 * HWO])
            return esum, e_sb

        def consume_back(votes_t, i, esum, e_sb, is_last):
            """Denominator matmul, softmax normalize, P2, out-accumulate."""
            if stage < 2:
                return
            votes_v = votes_t[:].rearrange("p (a r hw) -> p a r hw",
                                           a=4, r=8)
            den = qps.tile([32, 512], F32, tag="qk", name="den")[:, :HWO]
            nc.tensor.matmul(den[:], sd3_sb[:], esum[:],
                             start=True, stop=True)
            rd = mpool.tile([32, HWO], F32, tag="rd")
            nc.vector.reciprocal_approx_fast(rd[:], den[:])

            # ---- qk~ = E * rd (gpsimd); replicate over j into [128,(r,hw)]
            qkt = mpool.tile([32, 8 * HWO], BF16, tag="qkt")
            for r in range(8):
                nc.gpsimd.tensor_mul(
                    qkt[:, r * HWO:(r + 1) * HWO],
                    e_sb[:, r * HWO:(r + 1) * HWO], rd[:])
            if stage < 3:
                return
            rep = mpool.tile([128, 8 * HWO], BF16, tag="rep")
            for j in range(4):
                nc.sync.dma_start(rep[32 * j:32 * j + 32, :], qkt[:])

            # ---- P2 = votes * qk~rep
            p2 = p2pool.tile([128, 4 * 8 * HWO], BF16, tag="p2")
            p2_v = p2[:].rearrange("p (a r hw) -> p a r hw", a=4, r=8)
            for a in range(4):
                nc.vector.tensor_mul(
                    p2_v[:, a].rearrange("p r hw -> p (r hw)"),
                    votes_v[:, a].rearrange("p r hw -> p (r hw)"),
                    rep[:])

            # ---- out += SN_r^T @ P2
            if stage < 4:
                return
            for r in range(8):
                for h in range(2):
                    o0, onh = OHALVES[h]
                    nc.tensor.matmul(
                        oacc[h][:],
                        sn_sb[:].rearrange("p (r c) -> p r c", r=8)[:, r, :],
                        p2_v[:, :, r, o0:o0 + onh],
                        start=first_acc[h], stop=(is_last and r == 7),
                    )
                    first_acc[h] = False

        # 2-stage software pipeline: PE order per step t is
        # votes(t) / front(t-1) / back(t-2) so the den matmul never waits
        # on the gpsimd tree and the PE always has independent work
        steps = [(k, l, 3 * k + l, i)
                 for (k, l) in kls for i in range(4)]
        fr = {}
        for t, (k, l, kl, i) in enumerate(steps):
            vt = emit_votes(k, l, kl, i)
            if t >= 1:
                pk, pl, pkl, pi = steps[t - 1]
                fr[t - 1] = (fr[t - 1][0], consume_front(fr[t - 1][0], pi))
            if t >= 2:
                vt2, (es, eb) = fr.pop(t - 2)
                _, _, _, qi = steps[t - 2]
                consume_back(vt2, qi, es, eb, False)
            fr[t] = (vt, None)
        T = len(steps)
        fr[T - 1] = (fr[T - 1][0], consume_front(fr[T - 1][0],
                                                 steps[T - 1][3]))
        for t in (T - 2, T - 1):
            vt2, (es, eb) = fr.pop(t)
            consume_back(vt2, steps[t][3], es, eb, t == T - 1)

        # ---------------- LayerNorm over (a,d) + output permute + store
        if stage < 4:
            ztile = spool.tile([32, HWO * 16], F32, tag="fin")
            nc.vector.memset(ztile[:], 0.0)
            nc.sync.dma_start(dram["out"][bi], ztile[:])
            continue
        s1 = mpool.tile([128, HWO], F32, tag="s1", bufs=1)
        s2 = mpool.tile([128, HWO], F32, tag="s2", bufs=1)
        sqt = spool.tile([128, 452], F32, tag="sqt")
        for h in range(2):
            o0, onh = OHALVES[h]
            nc.vector.tensor_reduce(
                s1[:, o0:o0 + onh],
                oacc[h][:].rearrange("p (a q) -> p q a", a=4),
                axis=mybir.AxisListType.X, op=mybir.AluOpType.add)
            nc.scalar.activation(sqt[:, :4 * onh], oacc[h][:],
                                 mybir.ActivationFunctionType.Square)
            nc.vector.tensor_reduce(
                s2[:, o0:o0 + onh],
                sqt[:, :4 * onh].rearrange("p (a q) -> p q a", a=4),
                axis=mybir.AxisListType.X, op=mybir.AluOpType.add)
        mu_ps = qps.tile([128, 512], F32, tag="qk", name="mu")[:, :HWO]
        nc.tensor.matmul(mu_ps[:], sm_sb[:], s1[:], start=True, stop=True)
        mu = mpool.tile([128, HWO], F32, tag="mu", bufs=1)
        nc.scalar.copy(mu[:], mu_ps[:])
        e2_ps = qps.tile([128, 512], F32, tag="qk", name="e2")[:, :HWO]
        nc.tensor.matmul(e2_ps[:], sm_sb[:], s2[:], start=True, stop=True)
        var = mpool.tile([128, HWO], F32, tag="var", bufs=1)
        # var = E[x^2] - mu^2:  (e2 - mu*mu)
        musq = mpool.tile([128, HWO], F32, tag="musq", bufs=1)
        nc.vector.tensor_mul(musq[:], mu[:], mu[:])
        nc.vector.tensor_sub(var[:], e2_ps[:], musq[:])
        sig = mpool.tile([128, HWO], F32, tag="sig", bufs=1)
        nc.scalar.activation(sig[:], var[:],
                             mybir.ActivationFunctionType.Sqrt,
                             bias=eps_sb[:, 0:1])
        rstd = mpool.tile([128, HWO], F32, tag="rstd", bufs=1)
        nc.vector.reciprocal_approx_fast(rstd[:], sig[:])

        outn = spool.tile([128, 4 * HWO], F32, tag="outn")  # (a, hw)
        t3 = mpool.tile([128, HWO], F32, tag="t3", bufs=1)
        for h in range(2):
            o0, onh = OHALVES[h]
            for a in range(4):
                nc.vector.tensor_sub(t3[:, :onh],
                                     oacc[h][:, a * onh:(a + 1) * onh],
                                     mu[:, o0:o0 + onh])
                nc.vector.tensor_mul(t3[:, :onh], t3[:, :onh],
                                     rstd[:, o0:o0 + onh])
                nc.vector.tensor_scalar(
                    outn[:, a * HWO + o0:a * HWO + o0 + onh],
                    t3[:, :onh], gc_sb[:, a:a + 1], bc_sb[:, a:a + 1],
                    op0=mybir.AluOpType.mult, op1=mybir.AluOpType.add)

        fin = spool.tile([32, HWO * 16], F32, tag="fin")
        fin_v = fin[:].rearrange("p (hw ad) -> p hw ad", ad=16)
        outn_v = outn[:].rearrange("p (a hw) -> p a hw", a=4)
        for d in range(4):
            for h in range(2):
                o0, onh = OHALVES[h]
                fp = qps.tile([128, 512], F32, tag="qk",
                              name="fp")[:32, :4 * onh]
                nc.tensor.matmul(
                    fp[:], sd4_sb[:].rearrange("p (d c) -> p d c", d=4)[:, d, :],
                    outn_v[:, :, o0:o0 + onh],
                    start=True, stop=True)
                nc.scalar.copy(
                    fin_v[:, o0:o0 + onh, :].rearrange(
                        "p hw (a x) -> p a hw x", a=4)[:, :, :, d],
                    fp[:].rearrange("p (a q) -> p a q", a=4))
        nc.sync.dma_start(dram["out"][bi], fin[:])


# ------------------------------------------------------------------ driver
def _build_nc(nb=NB, kls=None, stage=4):
    nc = bacc.Bacc("TRN2", target_bir_lowering=False, debug=False,
                   num_devices=NCORES)
    dram = {}
    dram["inputT"] = nc.dram_tensor("inputT", (nb, 128, 4096), BF16,
                                    kind="ExternalInput").ap()
    dram["u_scr"] = nc.dram_tensor("u_scr", (nb, 128, 4 * 8 * HWO), BF16,
                                   kind="ExternalInput").ap()
    dram["w_bd"] = nc.dram_tensor("w_bd", (9, 8, 128, 128), BF16,
                                  kind="ExternalInput").ap()
    dram["sq"] = nc.dram_tensor("sq", (128, 32), BF16,
                                kind="ExternalInput").ap()
    dram["sd3"] = nc.dram_tensor("sd3", (32, 32), BF16,
                                 kind="ExternalInput").ap()
    dram["sn"] = nc.dram_tensor("sn", (8, 128, 128), BF16,
                                kind="ExternalInput").ap()
    dram["sm"] = nc.dram_tensor("sm", (128, 128), F32,
                                kind="ExternalInput").ap()
    dram["sd4"] = nc.dram_tensor("sd4", (4, 128, 32), F32,
                                 kind="ExternalInput").ap()
    dram["gcols"] = nc.dram_tensor("gcols", (128, 4), F32,
                                   kind="ExternalInput").ap()
    dram["bcols"] = nc.dram_tensor("bcols", (128, 4), F32,
                                   kind="ExternalInput").ap()
    dram["out"] = nc.dram_tensor("out", (nb, 32, HWO * 16), F32,
                                 kind="ExternalOutput").ap()
    with tile.TileContext(nc) as tc:
        build_program(tc, dram, nb=nb, kls=kls, stage=stage)
    nc.compile()
    return nc


def _run(trace, **inputs):
    input_ = np.asarray(inputs["input"], dtype=np.float32)
    ncv = np.asarray(inputs["next_capsule_value"], dtype=np.float32)
    w = np.asarray(inputs["w"], dtype=np.float32)
    gamma = np.asarray(inputs["gamma"], dtype=np.float32)
    beta = np.asarray(inputs["beta"], dtype=np.float32)

    hp = host_prep(input_, ncv, w, gamma, beta)
    nc = _build_nc()

    shared = {k: hp[k] for k in
              ("w_bd", "sq", "sd3", "sn", "sm", "sd4", "gcols", "bcols")}
    in_maps = []
    for c in range(NCORES):
        im = dict(shared)
        im["inputT"] = np.ascontiguousarray(hp["inputT"][c * NB:(c + 1) * NB])
        im["u_scr"] = np.ascontiguousarray(hp["u_scr"][c * NB:(c + 1) * NB])
        in_maps.append(im)

    from concourse.bass_utils import run_bass_kernel_spmd
    res = run_bass_kernel_spmd(nc, in_maps, core_ids=list(range(NCORES)),
                               trace=trace)
    outs = res.results
    full = np.concatenate([np.asarray(o["out"]) for o in outs], axis=0)
    return full.reshape(B, M, HO, WO, DOUT).astype(np.float32), res


def kernel(**inputs):
    return _run(False, **inputs)[0]


def kernel_traced(**inputs):
    return _run(True, **inputs)


def kernel_timed(reps=20, **inputs):
    """Run once for outputs + time repeated device executions (min over
    reps, inputs resident on device, excludes h2d)."""
    import time
    import jax
    from jax.experimental.shard_map import shard_map
    from jax.sharding import Mesh, PartitionSpec
    from concourse import bass2jax, mybir as _mb

    input_ = np.asarray(inputs["input"], dtype=np.float32)
    ncv = np.asarray(inputs["next_capsule_value"], dtype=np.float32)
    w = np.asarray(inputs["w"], dtype=np.float32)
    gamma = np.asarray(inputs["gamma"], dtype=np.float32)
    beta = np.asarray(inputs["beta"], dtype=np.float32)
    hp = host_prep(input_, ncv, w, gamma, beta)
    nc = _build_nc()
    shared = {k: hp[k] for k in
              ("w_bd", "sq", "sd3", "sn", "sm", "sd4", "gcols", "bcols")}
    in_maps = []
    for c in range(NCORES):
        im = dict(shared)
        im["inputT"] = np.ascontiguousarray(hp["inputT"][c * NB:(c + 1) * NB])
        im["u_scr"] = np.ascontiguousarray(hp["u_scr"][c * NB:(c + 1) * NB])
        in_maps.append(im)

    bass2jax.install_neuronx_cc_hook()
    partition_name = (nc.partition_id_tensor.name
                      if nc.partition_id_tensor else None)
    in_names, out_names, out_avals, zero_outs = [], [], [], []
    for alloc in nc.m.functions[0].allocations:
        if not isinstance(alloc, _mb.MemoryLocationSet):
            continue
        name = alloc.memorylocations[0].name
        if alloc.kind == "ExternalInput":
            if name != partition_name:
                in_names.append(name)
        elif alloc.kind == "ExternalOutput":
            shape = tuple(alloc.tensor_shape)
            dtype = _mb.dt.np(alloc.dtype)
            out_names.append(name)
            out_avals.append(jax.core.ShapedArray(shape, dtype))
            zero_outs.append(np.zeros(shape, dtype))
    n_params = len(in_names)
    all_in_names = list(in_names) + list(out_names)
    if partition_name is not None:
        all_in_names.append(partition_name)

    def _body(*args):
        operands = list(args)
        if partition_name is not None:
            operands.append(bass2jax.partition_id_tensor())
        outs = bass2jax._bass_exec_p.bind(
            *operands, out_avals=tuple(out_avals),
            in_names=tuple(all_in_names), out_names=tuple(out_names),
            lowering_input_output_aliases=(),
            sim_require_finite=True, sim_require_nnan=True, nc=nc)
        return tuple(outs)

    devices = jax.devices()[:NCORES]
    mesh = Mesh(np.asarray(devices), ("core",))
    n_outs = len(out_names)
    sharded = jax.jit(
        shard_map(_body, mesh=mesh,
                  in_specs=(PartitionSpec("core"),) * (n_params + n_outs),
                  out_specs=(PartitionSpec("core"),) * n_outs,
                  check_rep=False),
        keep_unused=True)
    concat_in = [np.concatenate([np.asarray(in_maps[c][nm])
                                 for c in range(NCORES)], axis=0)
                 for nm in in_names]
    concat_zeros = [np.zeros((NCORES * z.shape[0], *z.shape[1:]), z.dtype)
                    for z in zero_outs]
    def _chain(n):
        def f(*args):
            ins = list(args[:n_params])
            zs = list(args[n_params:])
            outs = zs
            for _ in range(n):
                outs = list(_body(*ins, *outs))
            return tuple(outs)
        return jax.jit(
            shard_map(f, mesh=mesh,
                      in_specs=(PartitionSpec("core"),) * (n_params + n_outs),
                      out_specs=(PartitionSpec("core"),) * n_outs,
                      check_rep=False),
            keep_unused=True)

    NCH = 8
    chain_f = _chain(NCH)
    dev_in = [jax.device_put(a) for a in concat_in]
    dev_zero = [jax.device_put(a) for a in concat_zeros]
    outs = sharded(*dev_in, *dev_zero)
    jax.block_until_ready(outs)
    co = chain_f(*dev_in, *dev_zero)
    jax.block_until_ready(co)
    t1s, tns = [], []
    for _ in range(reps):
        t0 = time.perf_counter()
        o1 = sharded(*dev_in, *dev_zero)
        jax.block_until_ready(o1)
        t1s.append(time.perf_counter() - t0)
        t0 = time.perf_counter()
        on = chain_f(*dev_in, *dev_zero)
        jax.block_until_ready(on)
        tns.append(time.perf_counter() - t0)
    t1s, tns = np.array(t1s), np.array(tns)
    slope = (np.median(tns) - np.median(t1s)) / (NCH - 1)
    slope_min = (tns.min() - t1s.min()) / (NCH - 1)
    out_full = np.asarray(outs[out_names.index("out")]).reshape(
        NCORES, NB, 32, HWO * 16)
    full = out_full.reshape(B, M, HO, WO, DOUT).astype(np.float32)
    return full, dict(t1=t1s, tn=tns, nch=NCH,
                      slope_ns=slope * 1e9, slope_min_ns=slope_min * 1e9)


if __name__ == "__main__":
    # smoke: build the program only
    nc = _build_nc()
    print("built OK, instructions:",
          sum(1 for _ in nc.m.functions[0].instructions)
          if hasattr(nc.m.functions[0], "instructions") else "?")



# revision 19
# speedup vs baseline: 1.6286x; 1.6286x over previous
"""CapsuleCONV Trainium2 kernel (nn_CapsuleCONV_1709396984016).

Math (per batch b):
  unfold input [N,32,32,16] with K=3,stride=2 -> patches X[n,k,l,hw,a,x]
  votes V[n,kl,hw,a,(d,m)] = sum_x X * w[k,l,n,x,d,m]
  logits qk[n,kl,m,hw] = 0.25 * sum_{a,d} V * ncv[b,m,hw,a,d]
  qk = softmax_m(qk);  out[m,hw,a,d] = sum_{n,kl} qk * V
  out = LayerNorm_{(a,d)}(out) * gamma + beta

Device mapping (8 cores, 4 batches each):
  partitions p = 4n+x = 32i+4g+x   (n = 8i+g)
  votes matmul: 16-tile-packed block-diagonal lhsT tiles (K=32 = 8 caps x
  4 pose cols on the diagonal), rhs = pose columns of the input slab.
  dm slot of PSUM partition (j,g,c4) at round r:  d=j, m=4r+c4.
  Selection matmuls (host-built 0/1 matrices) do all partition-dim sums:
  SQ (sum_j -> logits), SD (sum_{m} -> softmax denom, replicated),
  SN_r (sum_g, unscramble to 4m+d), SM (sum_d for LN, replicated),
  Sd4 (split d for the output permute).
"""
import os
import numpy as np
import ml_dtypes
from contextlib import ExitStack

import concourse.bass as bass
import concourse.tile as tile
from concourse import bacc, mybir
from concourse._compat import with_exitstack

F32 = mybir.dt.float32
BF16 = mybir.dt.bfloat16
BF = ml_dtypes.bfloat16

B, N, H, W, DIN = 32, 32, 32, 32, 16
M, DOUT = 32, 16
KK, STRIDE = 3, 2
HO = WO = 15
HWO = HO * WO  # 225
NCORES = 8
NB = B // NCORES  # 4 batches per core
SCALE = 0.25      # 1/sqrt(16)
LN_EPS = 1e-5
# h-halves for 512-elem PSUM banks: h rows [0,8) and [8,15)
HALVES = [(0, 8), (8, 7)]
# hw halves for the output accumulator (<=128 free per a)
OHALVES = [(0, 113), (113, 112)]


# ---------------------------------------------------------------- host prep
def host_prep(input_, ncv, w, gamma, beta):
    # [B, (n,x), (a, H, W)] — a OUTER of (h,w) so PSUM banks drain
    # contiguously (bank free = (a, hw) with hw innermost)
    inputT = np.ascontiguousarray(
        input_.reshape(B, N, H, W, 4, 4).transpose(0, 1, 5, 4, 2, 3)
        .reshape(B, 128, 4 * H * W)).astype(BF)

    # U_scr[b, 32j+4g+c4, a, r, hw] = ncv[b, 4r+c4, hw, a*4+j]
    nc6 = ncv.reshape(B, 8, 4, HWO, 4, 4)        # [b, r, c4, hw, a, d]
    u = nc6.transpose(0, 5, 2, 4, 1, 3)          # [b, d(=j), c4, a, r, hw]
    u = np.broadcast_to(u[:, :, None, :, :, :, :],
                        (B, 4, 8, 4, 4, 8, HWO))  # + g
    # -> [b, (j,g,c4), a, r, hw]
    u_scr = np.ascontiguousarray(
        u.transpose(0, 1, 2, 3, 4, 5, 6).reshape(B, 128, 4 * 8 * HWO)
    ).astype(BF)

    # w_bd[kl, r, 32i+4g+x, 32j+4g+c4] = w[k,l, 8i+g, x, j, 4r+c4]
    v = w.reshape(9, 4, 8, 4, 4, 8, 4)           # [kl, i, g, x, d(=j), r, c4]
    v = v.transpose(0, 5, 1, 2, 3, 4, 6)         # [kl, r, i, g, x, j, c4]
    eye8 = np.eye(8, dtype=np.float32)
    wbd = np.einsum('qrigxjc,gh->qrigxjhc', v, eye8)  # [kl,r,i,g,x,j,g',c4]
    w_bd = np.ascontiguousarray(wbd.reshape(9, 8, 128, 128)).astype(BF)

    # SQ[(j,g,c4), (g',c4')] = 0.25 * d_{g,g'} d_{c4,c4'}
    sq = np.zeros((128, 32), np.float32)
    for j in range(4):
        for g in range(8):
            for c in range(4):
                sq[32 * j + 4 * g + c, 4 * g + c] = SCALE
    # SD[(e,g,c4), (e',g',c4')] = d_{g,g'}
    sd = np.zeros((64, 64), np.float32)
    for e in range(2):
        for g in range(8):
            for c in range(4):
                for e2 in range(2):
                    for c2 in range(4):
                        sd[32 * e + 4 * g + c, 32 * e2 + 4 * g + c2] = 1.0
    # SN[r][(j,g,c4), 4*(4r+c4)+j] = 1   (sum over g)
    sn = np.zeros((8, 128, 128), np.float32)
    for r in range(8):
        for j in range(4):
            for g in range(8):
                for c in range(4):
                    sn[r, 32 * j + 4 * g + c, 4 * (4 * r + c) + j] = 1.0
    # SM[(4m+d), (4m'+d')] = d_{m,m'} / 16
    sm = np.zeros((128, 128), np.float32)
    for m in range(32):
        for d in range(4):
            for d2 in range(4):
                sm[4 * m + d, 4 * m + d2] = 1.0 / 16.0
    # Sd4[d][(4m+dd), m'] = d_{m,m'} d_{dd,d}
    sd4 = np.zeros((4, 128, 32), np.float32)
    for d in range(4):
        for m in range(32):
            sd4[d, 4 * m + d, m] = 1.0
    # per-partition gamma/beta columns: gcols[p, a] = gamma[a*4 + p%4]
    gcols = np.zeros((128, 4), np.float32)
    bcols = np.zeros((128, 4), np.float32)
    for p in range(128):
        for a in range(4):
            gcols[p, a] = gamma[a * 4 + p % 4]
            bcols[p, a] = beta[a * 4 + p % 4]

    return dict(inputT=inputT, u_scr=u_scr, w_bd=w_bd,
                sq=sq.astype(BF), sd=sd.astype(BF), sn=sn.astype(BF),
                sm=sm, sd4=sd4, gcols=gcols, bcols=bcols)


# ------------------------------------------------------------- tile program
@with_exitstack
def build_program(ctx: ExitStack, tc: tile.TileContext, dram: dict,
                  nb=NB, kls=None, stage=4):
    nc = tc.nc
    if kls is None:
        kls = [(k, l) for k in range(3) for l in range(3)]

    const = ctx.enter_context(tc.tile_pool(name="const", bufs=1))
    vpool = ctx.enter_context(tc.tile_pool(name="vpool", bufs=3))
    xpool = ctx.enter_context(tc.tile_pool(name="xpool", bufs=2))
    upool = ctx.enter_context(tc.tile_pool(name="upool", bufs=1))
    ppool = ctx.enter_context(tc.tile_pool(name="ppool", bufs=2))
    p2pool = ctx.enter_context(tc.tile_pool(name="p2pool", bufs=2))
    spool = ctx.enter_context(tc.tile_pool(name="spool", bufs=1))
    mpool = ctx.enter_context(tc.tile_pool(name="mpool", bufs=2))
    vps = ctx.enter_context(tc.tile_pool(name="vps", bufs=2, space="PSUM"))
    qps = ctx.enter_context(tc.tile_pool(name="qps", bufs=2, space="PSUM"))
    ops = ctx.enter_context(tc.tile_pool(name="ops", bufs=1, space="PSUM"))

    # constants
    wbd_sb = const.tile([128, 9 * 8 * 128], BF16, tag="wbd")
    nc.sync.dma_start(wbd_sb[:].rearrange("p (q r c) -> p q r c", q=9, r=8),
                      dram["w_bd"].rearrange("q r p c -> p q r c"))
    sq_sb = const.tile([128, 32], BF16, tag="sq")
    nc.sync.dma_start(sq_sb[:], dram["sq"])
    sd_sb = const.tile([64, 64], BF16, tag="sd")
    nc.sync.dma_start(sd_sb[:], dram["sd"])
    sn_sb = const.tile([128, 8 * 128], BF16, tag="sn")
    nc.sync.dma_start(sn_sb[:].rearrange("p (r c) -> p r c", r=8),
                      dram["sn"].rearrange("r p c -> p r c"))
    sm_sb = const.tile([128, 128], F32, tag="sm")
    nc.sync.dma_start(sm_sb[:], dram["sm"])
    sd4_sb = const.tile([128, 4 * 32], F32, tag="sd4")
    nc.sync.dma_start(sd4_sb[:].rearrange("p (d c) -> p d c", d=4),
                      dram["sd4"].rearrange("d p c -> p d c"))
    gc_sb = const.tile([128, 4], F32, tag="gc")
    nc.sync.dma_start(gc_sb[:], dram["gcols"])
    bc_sb = const.tile([128, 4], F32, tag="bc")
    nc.sync.dma_start(bc_sb[:], dram["bcols"])
    eps_sb = const.tile([128, 1], F32, tag="eps")
    nc.vector.memset(eps_sb[:], LN_EPS)

    wbd_v = wbd_sb[:].rearrange("p (q r c) -> p q r c", q=9, r=8)

    for bi in range(nb):
        xt = xpool.tile([128, 4096], BF16, tag="xt")
        nc.sync.dma_start(xt[:], dram["inputT"][bi])
        xt_v = xt[:].rearrange("p (a h w) -> p a h w", a=4, h=32)
        u_sb = upool.tile([128, 4 * 8 * HWO], BF16, tag="u")
        nc.sync.dma_start(u_sb[:], dram["u_scr"][bi])

        # output accumulator: free = (a, hw-half)
        if stage >= 4:
            oacc = [ops.tile([128, 512], F32, tag=f"oacc{h}", name=f"oacc{h}")
                    [:, :4 * OHALVES[h][1]] for h in range(2)]
        first_acc = [True, True]
        dcnt = [0]

        def emit_votes(k, l, kl, i):
            """Votes for one (kl, i): r-paired 2-bank PSUM tiles, one
            contiguous drain per pair, alternating Scalar/GpSimd."""
            votes_t = vpool.tile([128, 4 * 8 * HWO], BF16, tag="votes")
            votes_v = votes_t[:].rearrange("p (a r hw) -> p a r hw", a=4, r=8)
            for rp in range(4):
                for hx, (h0, nh) in enumerate(HALVES):
                    pair = vps.tile([128, 1024], F32, tag="vb",
                                    name=f"vb{hx}")
                    for e in range(2):
                        r = 2 * rp + e
                        nc.tensor.matmul(
                            pair[:, 512 * e:512 * e + nh * 60],
                            wbd_v[32 * i:32 * i + 32, kl, r, :],
                            xt_v[32 * i:32 * i + 32, :,
                                 k + 2 * h0:k + 2 * (h0 + nh):2,
                                 l:l + 29:2],
                            start=True, stop=True,
                            tile_position=(32 * i, 0),
                        )
                    if stage >= 1:
                        src = (pair[:].rearrange("p (b x) -> p b x", b=2)
                               [:, :, :nh * 60]
                               .rearrange("p b (a q) -> p a b q", a=4))
                        dst = votes_v[:, :, 2 * rp:2 * rp + 2,
                                      h0 * 15:(h0 + nh) * 15]
                        if dcnt[0] % 4 == 3:
                            nc.vector.tensor_copy(dst, src)
                        else:
                            nc.scalar.copy(dst, src)
                        dcnt[0] += 1
            return votes_t

        def consume(votes_t, i, is_last):
            if stage < 2:
                return
            votes_v = votes_t[:].rearrange("p (a r hw) -> p a r hw",
                                           a=4, r=8)
            # ---- P = votes * U  (one TT, bf16 2x)
            pt = ppool.tile([128, 4 * 8 * HWO], BF16, tag="pt")
            nc.vector.tensor_mul(pt[:], votes_t[:], u_sb[:])
            pt_v = pt[:].rearrange("p (a r hw) -> p a r hw", a=4, r=8)

            # ---- logits: sum_j via SQ, sum_a via 4 accumulating MMs
            e_sb = spool.tile([64, 4 * HWO], BF16, tag="e", bufs=2,
                              name=f"e{i}")
            for r2 in range(4):
                qk = qps.tile([64, 512], F32, tag="qk",
                              name="qk")[:, :HWO]
                for e in range(2):
                    r = 2 * r2 + e
                    for a in range(4):
                        nc.tensor.matmul(
                            qk[32 * e:32 * e + 32, :],
                            sq_sb[:],
                            pt_v[:, a, r, :],
                            start=(a == 0), stop=(a == 3),
                        )
                nc.scalar.activation(
                    e_sb[:, r2 * HWO:(r2 + 1) * HWO], qk[:],
                    mybir.ActivationFunctionType.Exp)

            # ---- softmax denom (replicated) + reciprocal
            den = qps.tile([64, 512], F32, tag="qk", name="den")[:, :HWO]
            for r2 in range(4):
                nc.tensor.matmul(
                    den[:], sd_sb[:],
                    e_sb[:, r2 * HWO:(r2 + 1) * HWO],
                    start=(r2 == 0), stop=(r2 == 3))
            rd = mpool.tile([64, HWO], F32, tag="rd")
            nc.vector.reciprocal_approx_fast(rd[:], den[:])

            # ---- qk~ = E * rd (gpsimd); replicate over j into [128,(r,hw)]
            qkt = mpool.tile([64, 4 * HWO], BF16, tag="qkt")
            for r2 in range(4):
                nc.gpsimd.tensor_mul(
                    qkt[:, r2 * HWO:(r2 + 1) * HWO],
                    e_sb[:, r2 * HWO:(r2 + 1) * HWO], rd[:])
            if stage < 3:
                return
            rep = mpool.tile([128, 8 * HWO], BF16, tag="rep")
            rep_v = rep[:].rearrange("p (r2 e hw) -> p r2 e hw", r2=4, e=2)
            qkt_v = qkt[:].rearrange("p (r2 hw) -> p r2 hw", r2=4)
            for j in range(4):
                for e in range(2):
                    nc.sync.dma_start(
                        rep_v[32 * j:32 * j + 32, :, e, :],
                        qkt_v[32 * e:32 * e + 32])

            # ---- P2 = votes * qk~rep
            p2 = p2pool.tile([128, 4 * 8 * HWO], BF16, tag="p2")
            p2_v = p2[:].rearrange("p (a r hw) -> p a r hw", a=4, r=8)
            for a in range(4):
                nc.vector.tensor_mul(
                    p2_v[:, a].rearrange("p r hw -> p (r hw)"),
                    votes_v[:, a].rearrange("p r hw -> p (r hw)"),
                    rep[:])

            # ---- out += SN_r^T @ P2
            if stage < 4:
                return
            for r in range(8):
                for h in range(2):
                    o0, onh = OHALVES[h]
                    nc.tensor.matmul(
                        oacc[h][:],
                        sn_sb[:].rearrange("p (r c) -> p r c", r=8)[:, r, :],
                        p2_v[:, :, r, o0:o0 + onh],
                        start=first_acc[h], stop=(is_last and r == 7),
                    )
                    first_acc[h] = False

        # software pipeline: votes(t+1) emitted before consume(t) so the
        # PE has independent work while consume(t) waits on drains/DVE
        prev = None
        for ki, (k, l) in enumerate(kls):
            kl = 3 * k + l
            for i in range(4):
                vt = emit_votes(k, l, kl, i)
                if prev is not None:
                    consume(prev[0], prev[1], False)
                prev = (vt, i)
        consume(prev[0], prev[1], True)

        # ---------------- LayerNorm over (a,d) + output permute + store
        if stage < 4:
            ztile = spool.tile([32, HWO * 16], F32, tag="fin")
            nc.vector.memset(ztile[:], 0.0)
            nc.sync.dma_start(dram["out"][bi], ztile[:])
            continue
        s1 = mpool.tile([128, HWO], F32, tag="s1", bufs=1)
        s2 = mpool.tile([128, HWO], F32, tag="s2", bufs=1)
        sqt = spool.tile([128, 452], F32, tag="sqt")
        for h in range(2):
            o0, onh = OHALVES[h]
            nc.vector.tensor_reduce(
                s1[:, o0:o0 + onh],
                oacc[h][:].rearrange("p (a q) -> p q a", a=4),
                axis=mybir.AxisListType.X, op=mybir.AluOpType.add)
            nc.scalar.activation(sqt[:, :4 * onh], oacc[h][:],
                                 mybir.ActivationFunctionType.Square)
            nc.vector.tensor_reduce(
                s2[:, o0:o0 + onh],
                sqt[:, :4 * onh].rearrange("p (a q) -> p q a", a=4),
                axis=mybir.AxisListType.X, op=mybir.AluOpType.add)
        mu_ps = qps.tile([128, 512], F32, tag="qk", name="mu")[:, :HWO]
        nc.tensor.matmul(mu_ps[:], sm_sb[:], s1[:], start=True, stop=True)
        mu = mpool.tile([128, HWO], F32, tag="mu", bufs=1)
        nc.scalar.copy(mu[:], mu_ps[:])
        e2_ps = qps.tile([128, 512], F32, tag="qk", name="e2")[:, :HWO]
        nc.tensor.matmul(e2_ps[:], sm_sb[:], s2[:], start=True, stop=True)
        var = mpool.tile([128, HWO], F32, tag="var", bufs=1)
        # var = E[x^2] - mu^2:  (e2 - mu*mu)
        musq = mpool.tile([128, HWO], F32, tag="musq", bufs=1)
        nc.vector.tensor_mul(musq[:], mu[:], mu[:])
        nc.vector.tensor_sub(var[:], e2_ps[:], musq[:])
        sig = mpool.tile([128, HWO], F32, tag="sig", bufs=1)
        nc.scalar.activation(sig[:], var[:],
                             mybir.ActivationFunctionType.Sqrt,
                             bias=eps_sb[:, 0:1])
        rstd = mpool.tile([128, HWO], F32, tag="rstd", bufs=1)
        nc.vector.reciprocal_approx_fast(rstd[:], sig[:])

        outn = spool.tile([128, 4 * HWO], F32, tag="outn")  # (a, hw)
        t3 = mpool.tile([128, HWO], F32, tag="t3", bufs=1)
        for h in range(2):
            o0, onh = OHALVES[h]
            for a in range(4):
                nc.vector.tensor_sub(t3[:, :onh],
                                     oacc[h][:, a * onh:(a + 1) * onh],
                                     mu[:, o0:o0 + onh])
                nc.vector.tensor_mul(t3[:, :onh], t3[:, :onh],
                                     rstd[:, o0:o0 + onh])
                nc.vector.tensor_scalar(
                    outn[:, a * HWO + o0:a * HWO + o0 + onh],
                    t3[:, :onh], gc_sb[:, a:a + 1], bc_sb[:, a:a + 1],
                    op0=mybir.AluOpType.mult, op1=mybir.AluOpType.add)

        fin = spool.tile([32, HWO * 16], F32, tag="fin")
        fin_v = fin[:].rearrange("p (hw ad) -> p hw ad", ad=16)
        outn_v = outn[:].rearrange("p (a hw) -> p a hw", a=4)
        for d in range(4):
            for h in range(2):
                o0, onh = OHALVES[h]
                fp = qps.tile([128, 512], F32, tag="qk",
                              name="fp")[:32, :4 * onh]
                nc.tensor.matmul(
                    fp[:], sd4_sb[:].rearrange("p (d c) -> p d c", d=4)[:, d, :],
                    outn_v[:, :, o0:o0 + onh],
                    start=True, stop=True)
                nc.scalar.copy(
                    fin_v[:, o0:o0 + onh, :].rearrange(
                        "p hw (a x) -> p a hw x", a=4)[:, :, :, d],
                    fp[:].rearrange("p (a q) -> p a q", a=4))
        nc.sync.dma_start(dram["out"][bi], fin[:])


# ------------------------------------------------------------------ driver
def _build_nc(nb=NB, kls=None, stage=4):
    nc = bacc.Bacc("TRN2", target_bir_lowering=False, debug=False,
                   num_devices=NCORES)
    dram = {}
    dram["inputT"] = nc.dram_tensor("inputT", (nb, 128, 4096), BF16,
                                    kind="ExternalInput").ap()
    dram["u_scr"] = nc.dram_tensor("u_scr", (nb, 128, 4 * 8 * HWO), BF16,
                                   kind="ExternalInput").ap()
    dram["w_bd"] = nc.dram_tensor("w_bd", (9, 8, 128, 128), BF16,
                                  kind="ExternalInput").ap()
    dram["sq"] = nc.dram_tensor("sq", (128, 32), BF16,
                                kind="ExternalInput").ap()
    dram["sd"] = nc.dram_tensor("sd", (64, 64), BF16,
                                kind="ExternalInput").ap()
    dram["sn"] = nc.dram_tensor("sn", (8, 128, 128), BF16,
                                kind="ExternalInput").ap()
    dram["sm"] = nc.dram_tensor("sm", (128, 128), F32,
                                kind="ExternalInput").ap()
    dram["sd4"] = nc.dram_tensor("sd4", (4, 128, 32), F32,
                                 kind="ExternalInput").ap()
    dram["gcols"] = nc.dram_tensor("gcols", (128, 4), F32,
                                   kind="ExternalInput").ap()
    dram["bcols"] = nc.dram_tensor("bcols", (128, 4), F32,
                                   kind="ExternalInput").ap()
    dram["out"] = nc.dram_tensor("out", (nb, 32, HWO * 16), F32,
                                 kind="ExternalOutput").ap()
    with tile.TileContext(nc) as tc:
        build_program(tc, dram, nb=nb, kls=kls, stage=stage)
    nc.compile()
    return nc


def _run(trace, **inputs):
    input_ = np.asarray(inputs["input"], dtype=np.float32)
    ncv = np.asarray(inputs["next_capsule_value"], dtype=np.float32)
    w = np.asarray(inputs["w"], dtype=np.float32)
    gamma = np.asarray(inputs["gamma"], dtype=np.float32)
    beta = np.asarray(inputs["beta"], dtype=np.float32)

    hp = host_prep(input_, ncv, w, gamma, beta)
    nc = _build_nc()

    shared = {k: hp[k] for k in
              ("w_bd", "sq", "sd", "sn", "sm", "sd4", "gcols", "bcols")}
    in_maps = []
    for c in range(NCORES):
        im = dict(shared)
        im["inputT"] = np.ascontiguousarray(hp["inputT"][c * NB:(c + 1) * NB])
        im["u_scr"] = np.ascontiguousarray(hp["u_scr"][c * NB:(c + 1) * NB])
        in_maps.append(im)

    from concourse.bass_utils import run_bass_kernel_spmd
    res = run_bass_kernel_spmd(nc, in_maps, core_ids=list(range(NCORES)),
                               trace=trace)
    outs = res.results
    full = np.concatenate([np.asarray(o["out"]) for o in outs], axis=0)
    return full.reshape(B, M, HO, WO, DOUT).astype(np.float32), res


def kernel(**inputs):
    return _run(False, **inputs)[0]


def kernel_traced(**inputs):
    return _run(True, **inputs)


def kernel_timed(reps=20, **inputs):
    """Run once for outputs + time repeated device executions (min over
    reps, inputs resident on device, excludes h2d)."""
    import time
    import jax
    from jax.experimental.shard_map import shard_map
    from jax.sharding import Mesh, PartitionSpec
    from concourse import bass2jax, mybir as _mb

    input_ = np.asarray(inputs["input"], dtype=np.float32)
    ncv = np.asarray(inputs["next_capsule_value"], dtype=np.float32)
    w = np.asarray(inputs["w"], dtype=np.float32)
    gamma = np.asarray(inputs["gamma"], dtype=np.float32)
    beta = np.asarray(inputs["beta"], dtype=np.float32)
    hp = host_prep(input_, ncv, w, gamma, beta)
    nc = _build_nc()
    shared = {k: hp[k] for k in
              ("w_bd", "sq", "sd", "sn", "sm", "sd4", "gcols", "bcols")}
    in_maps = []
    for c in range(NCORES):
        im = dict(shared)
        im["inputT"] = np.ascontiguousarray(hp["inputT"][c * NB:(c + 1) * NB])
        im["u_scr"] = np.ascontiguousarray(hp["u_scr"][c * NB:(c + 1) * NB])
        in_maps.append(im)

    bass2jax.install_neuronx_cc_hook()
    partition_name = (nc.partition_id_tensor.name
                      if nc.partition_id_tensor else None)
    in_names, out_names, out_avals, zero_outs = [], [], [], []
    for alloc in nc.m.functions[0].allocations:
        if not isinstance(alloc, _mb.MemoryLocationSet):
            continue
        name = alloc.memorylocations[0].name
        if alloc.kind == "ExternalInput":
            if name != partition_name:
                in_names.append(name)
        elif alloc.kind == "ExternalOutput":
            shape = tuple(alloc.tensor_shape)
            dtype = _mb.dt.np(alloc.dtype)
            out_names.append(name)
            out_avals.append(jax.core.ShapedArray(shape, dtype))
            zero_outs.append(np.zeros(shape, dtype))
    n_params = len(in_names)
    all_in_names = list(in_names) + list(out_names)
    if partition_name is not None:
        all_in_names.append(partition_name)

    def _body(*args):
        operands = list(args)
        if partition_name is not None:
            operands.append(bass2jax.partition_id_tensor())
        outs = bass2jax._bass_exec_p.bind(
            *operands, out_avals=tuple(out_avals),
            in_names=tuple(all_in_names), out_names=tuple(out_names),
            lowering_input_output_aliases=(),
            sim_require_finite=True, sim_require_nnan=True, nc=nc)
        return tuple(outs)

    devices = jax.devices()[:NCORES]
    mesh = Mesh(np.asarray(devices), ("core",))
    n_outs = len(out_names)
    sharded = jax.jit(
        shard_map(_body, mesh=mesh,
                  in_specs=(PartitionSpec("core"),) * (n_params + n_outs),
                  out_specs=(PartitionSpec("core"),) * n_outs,
                  check_rep=False),
        keep_unused=True)
    concat_in = [np.concatenate([np.asarray(in_maps[c][nm])
                                 for c in range(NCORES)], axis=0)
                 for nm in in_names]
    concat_zeros = [np.zeros((NCORES * z.shape[0], *z.shape[1:]), z.dtype)
                    for z in zero_outs]
    def _chain(n):
        def f(*args):
            ins = list(args[:n_params])
            zs = list(args[n_params:])
            outs = zs
            for _ in range(n):
                outs = list(_body(*ins, *outs))
            return tuple(outs)
        return jax.jit(
            shard_map(f, mesh=mesh,
                      in_specs=(PartitionSpec("core"),) * (n_params + n_outs),
                      out_specs=(PartitionSpec("core"),) * n_outs,
                      check_rep=False),
            keep_unused=True)

    NCH = 8
    chain_f = _chain(NCH)
    dev_in = [jax.device_put(a) for a in concat_in]
    dev_zero = [jax.device_put(a) for a in concat_zeros]
    outs = sharded(*dev_in, *dev_zero)
    jax.block_until_ready(outs)
    co = chain_f(*dev_in, *dev_zero)
    jax.block_until_ready(co)
    t1s, tns = [], []
    for _ in range(reps):
        t0 = time.perf_counter()
        o1 = sharded(*dev_in, *dev_zero)
        jax.block_until_ready(o1)
        t1s.append(time.perf_counter() - t0)
        t0 = time.perf_counter()
        on = chain_f(*dev_in, *dev_zero)
        jax.block_until_ready(on)
        tns.append(time.perf_counter() - t0)
    t1s, tns = np.array(t1s), np.array(tns)
    slope = (np.median(tns) - np.median(t1s)) / (NCH - 1)
    slope_min = (tns.min() - t1s.min()) / (NCH - 1)
    out_full = np.asarray(outs[out_names.index("out")]).reshape(
        NCORES, NB, 32, HWO * 16)
    full = out_full.reshape(B, M, HO, WO, DOUT).astype(np.float32)
    return full, dict(t1=t1s, tn=tns, nch=NCH,
                      slope_ns=slope * 1e9, slope_min_ns=slope_min * 1e9)


if __name__ == "__main__":
    # smoke: build the program only
    nc = _build_nc()
    print("built OK, instructions:",
          sum(1 for _ in nc.m.functions[0].instructions)
          if hasattr(nc.m.functions[0], "instructions") else "?")

